# revision 1
# baseline (speedup 1.0000x reference)
"""Self-contained Trainium2 Bass kernel for nn_MultiHeadAttention_46969762349562.

Full fp32 inputs -> full fp32 output, distributed over 8 NeuronCores in two
SPMD stages (all FLOPs on device; host only slices/casts/concats for staging):

  stage 1 (core = batch x 4-head group): Q/K/V projections (column-parallel
    weights, activations staged pre-transposed in bf16), flash-style attention
    with transposed score tiles [k_seq, q] so exp (ScalarE) feeds AV matmuls
    whose [V | ones] stationary yields the softmax denominator for free
    (scores here are O(+-5), so no max-subtraction is needed); projections are
    software-pipelined into the attention ACT-bound phase via an emission-order
    filler with explicit producer/consumer requirements.
  host reshard: head-sharded x^T -> row-sharded x^T (pure slicing).
  stage 2 (core = batch x 512-row chunk): softmax normalization (reciprocal +
    PE-broadcast of per-head denominators) and the output projection + bias.
"""
import sys
for p in ('/opt/trn_rl_repo', '/root/.axon_site/_ro/trn_rl_repo'):
    if p not in sys.path:
        sys.path.append(p)
import numpy as np
import ml_dtypes
import concourse.bass as bass
import concourse.bacc as bacc
import concourse.mybir as mybir
from concourse import tile
from concourse.bass_utils import run_bass_kernel_spmd
from contextlib import ExitStack

dt = mybir.dt
AF = mybir.ActivationFunctionType
BF16 = ml_dtypes.bfloat16

B = 2
S = 2048
D = 1024
H = 16
DH = 64
HL = 4            # heads per core (stage 1)
DG = HL * DH      # 256 dims per head-group
QP = 1024         # q-pass width in attention
NQP = S // QP     # 2
NKS = S // 128    # 16
NI = D // 128     # 8 contraction tiles
CHUNK = S // 4    # 512 rows per stage-2 core
N_CORES = 8


def build_stage1(n_cores=8):
    nc = bacc.Bacc("TRN2", target_bir_lowering=False, debug=False, num_devices=n_cores)
    xq = nc.declare_dram_parameter("xq", [D, S], dt.bfloat16, isOutput=False)
    xk = nc.declare_dram_parameter("xk", [D, S], dt.bfloat16, isOutput=False)
    xv = nc.declare_dram_parameter("xv", [D, S], dt.bfloat16, isOutput=False)
    wq = nc.declare_dram_parameter("wq", [D, DG], dt.bfloat16, isOutput=False)
    wk = nc.declare_dram_parameter("wk", [D, DG], dt.bfloat16, isOutput=False)
    wv = nc.declare_dram_parameter("wv", [D, DG], dt.bfloat16, isOutput=False)
    bqkvT = nc.declare_dram_parameter("bqkvT", [128, 4], dt.float32, isOutput=False)
    bvb = nc.declare_dram_parameter("bvb", [128, DG], dt.float32, isOutput=False)
    xo = nc.declare_dram_parameter("xo", [HL * 65, S], dt.bfloat16, isOutput=True)

    with tile.TileContext(nc) as tc, ExitStack() as ctx:
        const = ctx.enter_context(tc.tile_pool(name="const", bufs=1))
        # biasT: [128, 4] fp32 = (bq|bk) per partition-block p: col 2*t+p
        # bvb: [128, 256] fp32 = bv broadcast across partitions
        biasT_sb = const.tile([128, 4], dt.float32, name="biasT", tag="biasT")
        nc.sync.dma_start(biasT_sb[:], bqkvT[:, 0:4])
        bvb_sb = const.tile([128, DG], dt.float32, name="bvb", tag="bvb")
        nc.sync.dma_start(bvb_sb[:], bvb[:])

        xpool = ctx.enter_context(tc.tile_pool(name="x", bufs=1))
        wpool = ctx.enter_context(tc.tile_pool(name="w", bufs=1))

        # weight tiles first (small), then X^T tiles in the order compute needs them
        wq_t = [wpool.tile([128, DG], dt.bfloat16, name=f"wq{i}", tag=f"wq{i}") for i in range(NI)]
        wk_t = [wpool.tile([128, DG], dt.bfloat16, name=f"wk{i}", tag=f"wk{i}") for i in range(NI)]
        wv_t = [wpool.tile([128, DG], dt.bfloat16, name=f"wv{i}", tag=f"wv{i}") for i in range(NI)]
        xq_t = [xpool.tile([128, S], dt.bfloat16, name=f"xq{i}", tag=f"xq{i}") for i in range(NI)]
        xk_t = [xpool.tile([128, S], dt.bfloat16, name=f"xk{i}", tag=f"xk{i}") for i in range(NI)]
        xv_t = [xpool.tile([128, S], dt.bfloat16, name=f"xv{i}", tag=f"xv{i}") for i in range(NI)]
        for i in range(NI):
            nc.sync.dma_start(wk_t[i][:], wk[128 * i:128 * (i + 1), :])
        for i in range(NI):
            nc.sync.dma_start(xk_t[i][:], xk[128 * i:128 * (i + 1), :])
        for i in range(NI):
            nc.sync.dma_start(wq_t[i][:], wq[128 * i:128 * (i + 1), :])
        for i in range(NI):
            nc.sync.dma_start(xq_t[i][:], xq[128 * i:128 * (i + 1), :])
        for i in range(NI):
            nc.sync.dma_start(wv_t[i][:], wv[128 * i:128 * (i + 1), :])
        for i in range(NI):
            nc.sync.dma_start(xv_t[i][:], xv[128 * i:128 * (i + 1), :])

        qT = [xpool.tile([128, S], dt.bfloat16, name=f"qT{p}", tag=f"qT{p}") for p in range(2)]
        kT = [xpool.tile([128, S], dt.bfloat16, name=f"kT{p}", tag=f"kT{p}") for p in range(2)]
        vext = [xpool.tile([128, HL * 65], dt.bfloat16, name=f"vx{st}", tag=f"vx{st}") for st in range(NKS)]
        xo_sb = [xpool.tile([65, S], dt.bfloat16, name=f"xo{hl}", tag=f"xo{hl}") for hl in range(HL)]

        pp = ctx.enter_context(tc.tile_pool(name="pp", bufs=2, space="PSUM"))
        pj = ctx.enter_context(tc.tile_pool(name="pj", bufs=1, space="PSUM"))
        av_pool = ctx.enter_context(tc.tile_pool(name="av", bufs=1, space="PSUM"))
        exps = ctx.enter_context(tc.tile_pool(name="exps", bufs=12))

        def proj_T_units(xt, wt, dst, bias_col, p, label, halves=(0, 1)):
            """Generator: each yield boundary is ~0.5us of PE work.
            Bias (varies along the partition dim = output feature) is folded
            into the psum eviction as a per-partition tensor_scalar add."""
            bias_ap = biasT_sb[:, bias_col + p:bias_col + p + 1]
            for half in halves:
                ps = pj.tile([128, 1024], dt.float32, name="pjt", tag="pjt")
                for i in range(NI):
                    for qh in range(2):
                        qp4 = 2 * half + qh
                        nc.tensor.matmul(
                            ps[:, 512 * qh:512 * (qh + 1)],
                            lhsT=wt[i][:, 128 * p:128 * (p + 1)],
                            rhs=xt[i][:, 512 * qp4:512 * (qp4 + 1)],
                            start=(i == 0), stop=(i == NI - 1))
                    yield None
                nc.vector.tensor_scalar_add(
                    dst[:, 1024 * half:1024 * (half + 1)], ps[:], bias_ap)
                yield (label, half + 1)

        def proj_v_units(pr):
            # head-pair pr: produce vext columns for heads 2*pr, 2*pr+1.
            # st in groups of 4 sharing one psum tile; i-outer amortizes ldweights
            for grp in range(8):
                sts = [2 * grp + j for j in range(2)]
                ps = pj.tile([128, 1024], dt.float32, name="pjt", tag="pjt")
                for i in range(NI):
                    for j, st in enumerate(sts):
                        nc.tensor.matmul(
                            ps[:, 512 * j:512 * j + 128],
                            lhsT=xv_t[i][:, 128 * st:128 * (st + 1)],
                            rhs=wv_t[i][:, 128 * pr:128 * (pr + 1)],
                            start=(i == 0), stop=(i == NI - 1))
                    if i % 2 == 1:
                        yield None
                for j, st in enumerate(sts):
                    nc.vector.memset(vext[st][:, 130 * pr + 64:130 * pr + 65], 1.0)
                    nc.vector.memset(vext[st][:, 130 * pr + 129:130 * pr + 130], 1.0)
                    for hh in range(2):
                        hl = 2 * pr + hh
                        nc.vector.tensor_tensor(
                            vext[st][:, 65 * hl:65 * hl + 64],
                            ps[:, 512 * j + 64 * hh:512 * j + 64 * hh + 64],
                            bvb_sb[:, 64 * hl:64 * hl + 64],
                            mybir.AluOpType.add)
                    yield ((f"v{pr}", st + 1) if j == 1 else None)

        class Filler:
            """Pull-based emitter over a chain of labeled proj-unit generators.
            Progress labels mark completed (tensor, count) productions so the
            consumer can require producers to be emitted before readers."""
            def __init__(self, units):
                self.units = units
                self.done = {}
                self.empty = False

            def pull(self, n=1):
                for _ in range(n):
                    try:
                        lab = next(self.units)
                    except StopIteration:
                        self.empty = True
                        return
                    if lab is not None:
                        self.done[lab[0]] = lab[1]

            def require(self, key, count):
                while self.done.get(key, 0) < count:
                    assert not self.empty, f"filler exhausted before {key}={count}"
                    self.pull(1)

        def attention(p, fill, greedy_iters=0):
            # head-serial: per (qp of 1024, head): sweep kseq tiles; one exp call
            # per [128,1024] score tile; AV accumulates [65,1024] per head.
            it = 0
            for qp in range(NQP):
                for h in range(2):
                    hl = 2 * p + h
                    lo = 64 * h
                    if p == 0 and qp == 1:
                        fill.require("q0x", 2)
                    else:
                        fill.require(f"q{p}", qp + 1)
                    av = av_pool.tile([65, QP], dt.float32, name="av", tag="av")
                    for ks in range(NKS):
                        fill.require(f"k{p}", 1 if ks < 8 else 2)
                        qk = pp.tile([128, QP], dt.float32, name="qkt", tag="qkt")
                        for nh in range(2):
                            nc.tensor.matmul(
                                qk[:, 512 * nh:512 * (nh + 1)],
                                lhsT=kT[p][lo:lo + 64, 128 * ks:128 * (ks + 1)],
                                rhs=qT[p][lo:lo + 64, QP * qp + 512 * nh:QP * qp + 512 * nh + 512],
                                start=True, stop=True)
                        ex = exps.tile([128, QP], dt.bfloat16, name="ex", tag="ex")
                        nc.scalar.activation(ex[:], qk[:], AF.Exp, scale=0.125)
                        fill.require(f"v{p}", ks + 1)
                        for nh in range(2):
                            nc.tensor.matmul(
                                av[:, 512 * nh:512 * (nh + 1)],
                                lhsT=vext[ks][:, 65 * hl:65 * hl + 65],
                                rhs=ex[:, 512 * nh:512 * (nh + 1)],
                                start=(ks == 0), stop=(ks == NKS - 1))
                        fill.pull(2 if it < greedy_iters else 1)
                        it += 1
                    for nh in range(2):
                        nc.vector.tensor_copy(
                            xo_sb[hl][:, QP * qp + 512 * nh:QP * qp + 512 * (nh + 1)],
                            av[:, 512 * nh:512 * (nh + 1)])
                    nc.sync.dma_start(
                        xo[65 * hl:65 * hl + 65, QP * qp:QP * (qp + 1)],
                        xo_sb[hl][:, QP * qp:QP * (qp + 1)])

        from itertools import chain
        fill = Filler(chain(
            proj_T_units(xk_t, wk_t, kT[0], 2, 0, "k0"),
            proj_T_units(xq_t, wq_t, qT[0], 0, 0, "q0", halves=(0,)),
            proj_v_units(0),
            proj_T_units(xq_t, wq_t, qT[0], 0, 0, "q0x", halves=(1,)),
            proj_T_units(xq_t, wq_t, qT[1], 0, 1, "q1"),
            proj_T_units(xk_t, wk_t, kT[1], 2, 1, "k1"),
            proj_v_units(1),
        ))
        attention(0, fill, greedy_iters=8)
        attention(1, fill, greedy_iters=8)
        while not fill.empty:
            fill.pull(4)

    nc.compile()
    return nc


def build_stage2(n_cores=8):
    nc = bacc.Bacc("TRN2", target_bir_lowering=False, debug=False, num_devices=n_cores)
    xT = nc.declare_dram_parameter("xT", [D, CHUNK], dt.bfloat16, isOutput=False)
    dn = nc.declare_dram_parameter("dn", [H, CHUNK], dt.bfloat16, isOutput=False)
    woT = nc.declare_dram_parameter("woT", [D, D], dt.bfloat16, isOutput=False)
    bo = nc.declare_dram_parameter("bo", [1, D], dt.bfloat16, isOutput=False)
    sel = nc.declare_dram_parameter("sel", [H, D], dt.bfloat16, isOutput=False)
    out = nc.declare_dram_parameter("out", [CHUNK, D], dt.float32, isOutput=True)

    with tile.TileContext(nc) as tc, ExitStack() as ctx:
        const = ctx.enter_context(tc.tile_pool(name="const", bufs=1))
        ones_k1 = const.tile([1, 128], dt.bfloat16, name="ones_k1", tag="ones_k1")
        nc.vector.memset(ones_k1[:], 1.0)
        dn_sb = const.tile([H, CHUNK], dt.bfloat16, name="dn", tag="dn")
        nc.sync.dma_start(dn_sb[:], dn[:])
        sel_sb = const.tile([H, D], dt.bfloat16, name="sel", tag="sel")
        nc.sync.dma_start(sel_sb[:], sel[:])
        bo_sb = const.tile([1, D], dt.bfloat16, name="bo", tag="bo")
        nc.sync.dma_start(bo_sb[:], bo[:])

        xpool = ctx.enter_context(tc.tile_pool(name="x", bufs=1))
        xT_t = [xpool.tile([128, CHUNK], dt.bfloat16, name=f"xT{t}", tag=f"xT{t}") for t in range(NI)]
        wo_t = [xpool.tile([128, D], dt.bfloat16, name=f"wo{t}", tag=f"wo{t}") for t in range(NI)]
        for t in range(NI):
            nc.sync.dma_start(xT_t[t][:], xT[128 * t:128 * (t + 1), :])
            nc.sync.dma_start(wo_t[t][:], woT[128 * t:128 * (t + 1), :])

        rcp32 = const.tile([H, CHUNK], dt.float32, name="rcp32", tag="rcp32")
        rcp_bf = const.tile([H, CHUNK], dt.bfloat16, name="rcp_bf", tag="rcp_bf")
        nc.vector.reciprocal(rcp32[:], dn_sb[:])
        nc.vector.tensor_copy(rcp_bf[:], rcp32[:])

        ps_pool = ctx.enter_context(tc.tile_pool(name="ps", bufs=2, space="PSUM"))
        xn_t = [xpool.tile([128, CHUNK], dt.bfloat16, name=f"xn{t}", tag=f"xn{t}") for t in range(NI)]
        for t in range(NI):
            bc = ps_pool.tile([128, 512], dt.float32, name="bc", tag="bc")
            nc.tensor.matmul(bc[:], lhsT=sel_sb[:, 128 * t:128 * (t + 1)],
                             rhs=rcp_bf[:], start=True, stop=True)
            nc.vector.tensor_mul(xn_t[t][:], xT_t[t][:], bc[:])

        out_sb = [xpool.tile([128, D], dt.float32, name=f"os{st}", tag=f"os{st}") for st in range(4)]
        for st in range(4):
            for nh in range(2):
                ps = ps_pool.tile([128, 512], dt.float32, name=f"fc{nh}", tag=f"fc{nh}")
                for t in range(NI):
                    nc.tensor.matmul(
                        ps[:], lhsT=xn_t[t][:, 128 * st:128 * (st + 1)],
                        rhs=wo_t[t][:, 512 * nh:512 * (nh + 1)],
                        start=(t == 0), stop=False)
                nc.tensor.matmul(ps[:], lhsT=ones_k1[0:1, 0:128],
                                 rhs=bo_sb[0:1, 512 * nh:512 * (nh + 1)],
                                 start=False, stop=True)
                nc.vector.tensor_copy(out_sb[st][:, 512 * nh:512 * (nh + 1)], ps[:])
                nc.sync.dma_start(
                    out[128 * st:128 * (st + 1), 512 * nh:512 * (nh + 1)],
                    out_sb[st][:, 512 * nh:512 * (nh + 1)])

    nc.compile()
    return nc


def stage1_inputs(inputs):
    """inputs: dict with full query/key/value/W*/b* fp32. Returns in_maps for 8 cores.

    core c = (b, g): b = c // 4, g = c % 4 (head-group of 4 heads).
    """
    q = np.asarray(inputs['query'])
    k = np.asarray(inputs['key'])
    v = np.asarray(inputs['value'])
    maps = []
    for c in range(8):
        b, g = divmod(c, 4)
        rows = slice(DG * g, DG * (g + 1))
        bq = np.asarray(inputs['bq'])[rows].astype(np.float32)
        bk = np.asarray(inputs['bk'])[rows].astype(np.float32)
        bv = np.asarray(inputs['bv'])[rows].astype(np.float32)
        bqkvT = np.stack([bq[0:128], bq[128:256], bk[0:128], bk[128:256]], axis=1)
        bvb = np.broadcast_to(bv[None, :], (128, DG)).copy()
        maps.append({
            'xq': np.ascontiguousarray(q[b].T).astype(BF16),
            'xk': np.ascontiguousarray(k[b].T).astype(BF16),
            'xv': np.ascontiguousarray(v[b].T).astype(BF16),
            'wq': np.ascontiguousarray(np.asarray(inputs['Wq'])[rows].T).astype(BF16),
            'wk': np.ascontiguousarray(np.asarray(inputs['Wk'])[rows].T).astype(BF16),
            'wv': np.ascontiguousarray(np.asarray(inputs['Wv'])[rows].T).astype(BF16),
            'bqkvT': bqkvT, 'bvb': bvb,
        })
    return maps


def stage2_inputs(stage1_results, inputs):
    """stage1_results: list of 8 dicts with 'xo' [260, 2048] bf16."""
    woT = np.ascontiguousarray(np.asarray(inputs['Wo']).T).astype(BF16)
    bo = np.asarray(inputs['bo'])[None, :].astype(BF16)
    sel = np.zeros((H, D), dtype=BF16)
    for h in range(H):
        sel[h, DH * h:DH * (h + 1)] = 1.0
    # per batch: x^T [1024, 2048] and dn [16, 2048] from the 4 group cores
    maps = []
    for c in range(8):
        b, j = divmod(c, 4)
        cols = slice(CHUNK * j, CHUNK * (j + 1))
        xT = np.empty((D, CHUNK), dtype=BF16)
        dnm = np.empty((H, CHUNK), dtype=BF16)
        for g in range(4):
            xo = np.asarray(stage1_results[4 * b + g]['xo'])
            for hl in range(HL):
                hg = 4 * g + hl
                xT[DH * hg:DH * (hg + 1), :] = xo[65 * hl:65 * hl + 64, cols]
                dnm[hg, :] = xo[65 * hl + 64, cols]
        maps.append({'xT': xT, 'dn': dnm, 'woT': woT, 'bo': bo, 'sel': sel})
    return maps


def assemble_output(stage2_results):
    out = np.empty((B, S, D), dtype=np.float32)
    for c in range(8):
        b, j = divmod(c, 4)
        out[b, CHUNK * j:CHUNK * (j + 1), :] = np.asarray(stage2_results[c]['out'])
    return out

_CACHE = {}


def _programs():
    if 'nc1' not in _CACHE:
        _CACHE['nc1'] = build_stage1(N_CORES)
        _CACHE['nc2'] = build_stage2(N_CORES)
    return _CACHE['nc1'], _CACHE['nc2']


def kernel(**inputs):
    nc1, nc2 = _programs()
    core_ids = list(range(N_CORES))
    s1_maps = stage1_inputs(inputs)
    r1 = run_bass_kernel_spmd(nc1, s1_maps, core_ids).results
    s2_maps = stage2_inputs(r1, inputs)
    r2 = run_bass_kernel_spmd(nc2, s2_maps, core_ids).results
    return assemble_output(r2)



# revision 26
# speedup vs baseline: 1.2197x; 1.2197x over previous
"""Self-contained Trainium2 Bass kernel for nn_MultiHeadAttention_46969762349562.

Full fp32 inputs -> full fp32 output, distributed over 8 NeuronCores in two
SPMD stages (all FLOPs on device; host only slices/casts/concats/transposes):

  stage 1 (core = batch x 4-head group): Q/K/V projections (column-parallel
    weights, activations staged pre-transposed in bf16), attention with
    transposed score tiles [k_seq, q].  AV uses the transposed formulation
    out[q, vdim] = ex^T @ [V | ones]  (ex as the matmul stationary): 65-col
    AV matmuls at full PE utilization, half the PE cycles of the moving-ex
    form, with the softmax denominator free in column 64.  Scores are O(+-6)
    so no max-subtraction is needed.  exp runs on the ACT engine except a
    tunable subset of score tiles computed on DVE via a Schraudolph
    bit-trick (int16(x*128/ln2 + beta) bitcast to bf16), offloading the ACT
    bottleneck.  Softmax normalization happens in-stage on DVE (the
    denominator is a per-partition scalar in this layout).  Projections and
    the V/AV tail of the first pass are software-pipelined into later
    passes via an emission-order filler with just-in-time requirements;
    warmup matmuls keep the PE p-state hot through the DMA-bound startup.
  host reshard: head-sharded row-major x -> row-sharded x^T (slices+transpose).
  stage 2 (core = batch x 512-row chunk): output projection + bias only,
    group-outer accumulation so output DMA overlaps compute.
"""
import sys
for p in ('/opt/trn_rl_repo', '/root/.axon_site/_ro/trn_rl_repo'):
    if p not in sys.path:
        sys.path.append(p)
import numpy as np
import ml_dtypes
import concourse.bass as bass
import concourse.bacc as bacc
import concourse.mybir as mybir
from concourse import tile
from concourse.bass_utils import run_bass_kernel_spmd
from contextlib import ExitStack
from itertools import chain

dt = mybir.dt
AF = mybir.ActivationFunctionType
ALU = mybir.AluOpType
BF16 = ml_dtypes.bfloat16

B = 2
S = 2048
D = 1024
H = 16
DH = 64
HL = 4            # heads per core (stage 1)
DG = HL * DH      # 256 dims per head-group
NI = D // 128     # 8 contraction tiles
NKS = S // 128    # 16 key tiles
QP = 1024         # q-pass width
NQS = QP // 128   # 8 q sub-blocks per pass
CHUNK = S // 4    # 512 rows per stage-2 core
N_CORES = 8
NEX = 28          # exp-tile ring size (2 banks hold one open
                  # accumulation region each, so AV defers a full pass)

# Schraudolph exp on DVE: bf16bits(exp(x/8)) ~= int16(qk * A_DVE + B_DVE)
A_DVE = 128.0 / np.log(2.0) / 8.0
B_DVE = 16248.6               # 127*128 recentred for the sawtooth bias
ALT_PASSES = ()   # passes alternating exp between ACT and DVE (off: no gain)


def build_stage1(n_cores=8):
    nc = bacc.Bacc("TRN2", target_bir_lowering=False, debug=False, num_devices=n_cores)
    # [p, i, s] layouts: element (p, i, s) = x^T[128*i + p, s]
    xq = nc.declare_dram_parameter("xq", [128, NI, S], dt.bfloat16, isOutput=False)
    xk = nc.declare_dram_parameter("xk", [128, NI, S], dt.bfloat16, isOutput=False)
    xv = nc.declare_dram_parameter("xv", [128, NI, S], dt.bfloat16, isOutput=False)
    # wqkv[p, i, :] = (Wq.T | Wk.T | Wv.T)[128*i + p, :] (256 cols each)
    wqkv = nc.declare_dram_parameter("wqkv", [128, NI, 3 * DG], dt.bfloat16, isOutput=False)
    bqkT = nc.declare_dram_parameter("bqkT", [128, 4], dt.float32, isOutput=False)
    # bvb[p, r, :] = bv (same for all p, r): 4x-replicated for batched evictions
    bvb = nc.declare_dram_parameter("bvb", [128, 4, DG], dt.float32, isOutput=False)
    # normalized attention output: [qp*8+qs, p, c] = x[1024qp+128qs+p, c]
    xo = nc.declare_dram_parameter("xo", [16, 128, DG], dt.bfloat16, isOutput=True)

    with tile.TileContext(nc) as tc, ExitStack() as ctx:
        const = ctx.enter_context(tc.tile_pool(name="const", bufs=1))
        biasT = const.tile([128, 4], dt.float32, name="biasT", tag="biasT")
        bvb_sb = const.tile([128, 4, DG], dt.float32, name="bvb", tag="bvb")
        dum = const.tile([128, 512], dt.bfloat16, name="dum", tag="dum")
        nc.vector.memset(dum[:], 0.0)
        nc.sync.dma_start(biasT[:], bqkT[:])
        nc.sync.dma_start(bvb_sb[:], bvb[:])

        xpool = ctx.enter_context(tc.tile_pool(name="x", bufs=1))
        wq_sb = xpool.tile([128, NI, 3 * DG], dt.bfloat16, name="wq_sb", tag="w")
        xk_sb = xpool.tile([128, NI, S], dt.bfloat16, name="xk_sb", tag="xk")
        xq_sb = xpool.tile([128, NI, S], dt.bfloat16, name="xq_sb", tag="xq")
        xv_sb = xpool.tile([128, NI, S], dt.bfloat16, name="xv_sb", tag="xv")

        # staged input DMA: K weights+first keys, Q first window, then K rest
        # just-in-time for the exp pacing, V, and finally the second Q window
        nc.sync.dma_start(wq_sb[:, :, DG:2 * DG], wqkv[:, :, DG:2 * DG])
        nc.sync.dma_start(wq_sb[:, :, 0:DG], wqkv[:, :, 0:DG])
        nc.sync.dma_start(xk_sb[:, :, 0:512], xk[:, :, 0:512])
        nc.sync.dma_start(xq_sb[:, :, 0:512], xq[:, :, 0:512])
        nc.sync.dma_start(xq_sb[:, :, 512:1024], xq[:, :, 512:1024])
        nc.sync.dma_start(xk_sb[:, :, 512:1024], xk[:, :, 512:1024])
        nc.sync.dma_start(xk_sb[:, :, 1024:1536], xk[:, :, 1024:1536])
        nc.sync.dma_start(xk_sb[:, :, 1536:2048], xk[:, :, 1536:2048])
        nc.sync.dma_start(wq_sb[:, :, 2 * DG:3 * DG], wqkv[:, :, 2 * DG:3 * DG])
        nc.sync.dma_start(xv_sb[:, :, 0:QP], xv[:, :, 0:QP])
        nc.sync.dma_start(xv_sb[:, :, QP:S], xv[:, :, QP:S])
        nc.sync.dma_start(xq_sb[:, :, QP:S], xq[:, :, QP:S])

        kT = [xpool.tile([128, S], dt.bfloat16, name=f"kT{p}", tag=f"kT{p}") for p in range(2)]
        qT = [xpool.tile([128, S], dt.bfloat16, name=f"qT{p}", tag=f"qT{p}") for p in range(2)]
        vext = xpool.tile([128, NKS, HL, DH + 1], dt.bfloat16, name="vext", tag="vext")
        nc.vector.memset(vext[:, :, :, DH:DH + 1], 1.0)
        xo_big = [xpool.tile([128, NQS, DG], dt.bfloat16, name=f"xo{qp}", tag=f"xo{qp}")
                  for qp in range(2)]

        pp = ctx.enter_context(tc.tile_pool(name="pp", bufs=2, space="PSUM"))
        avp = ctx.enter_context(tc.tile_pool(name="avp", bufs=1, space="PSUM"))
        pj = ctx.enter_context(tc.tile_pool(name="pj", bufs=2, space="PSUM"))
        exps = ctx.enter_context(tc.tile_pool(name="exps", bufs=NEX))
        ex16s = ctx.enter_context(tc.tile_pool(name="ex16s", bufs=3))
        rcps = ctx.enter_context(tc.tile_pool(name="rcps", bufs=2))

        def gen_warmup(n, lhs_ap):
            # keeps the PE p-state hot; lhs_ap gates the block on a DMA
            for _ in range(n):
                ps = pj.tile([128, 512], dt.float32, name="pjt", tag="pjt")
                nc.tensor.matmul(ps[:], lhsT=lhs_ap, rhs=dum[:],
                                 start=True, stop=True)
                yield None

        def gen_proj_chunk(xt, dst, wcol, bias_col, label, c):
            # dst[:, 512c:512c+512] = (W block).T @ x chunk + bias
            ps = pj.tile([128, 512], dt.float32, name="pjt", tag="pjt")
            for i in range(NI):
                nc.tensor.matmul(
                    ps[:], lhsT=wq_sb[:, i, wcol:wcol + 128],
                    rhs=xt[:, i, 512 * c:512 * (c + 1)],
                    start=(i == 0), stop=(i == NI - 1))
                yield None
            nc.vector.tensor_scalar_add(
                dst[:, 512 * c:512 * (c + 1)], ps[:],
                biasT[:, bias_col:bias_col + 1])
            yield (label, c + 1)

        # per-pass state shared with the deferred tail generators
        tails = {}

        def normalize(av, hl, qp, mix=False):
            for lohi in range(2):
                rcp = rcps.tile([128, 4], dt.float32, name="rcp", tag="rcp")
                nc.vector.reciprocal(rcp[:], av[lohi][:, :, DH:DH + 1])
                for q4 in range(4):
                    qs = 4 * lohi + q4
                    dst = xo_big[qp][:, qs, DH * hl:DH * (hl + 1)]
                    src = av[lohi][:, q4, 0:DH]
                    sc = rcp[:, q4:q4 + 1]
                    if mix and q4 % 2 == 1:
                        nc.scalar.mul(dst, src, sc)
                    else:
                        nc.vector.tensor_scalar_mul(dst, src, sc)
                if hl == HL - 1:
                    # dst AP reordered to (p, qs, c) so the SBUF source AP
                    # keeps its partition dim first (walrus requirement)
                    nc.sync.dma_start(
                        xo[8 * qp + 4 * lohi:8 * qp + 4 * lohi + 4, :, :]
                        .transpose([1, 0, 2]),
                        xo_big[qp][:, 4 * lohi:4 * lohi + 4, :])

        def v_unit4(hl, kb):
            # vext[:, 4kb:4kb+4, hl, 0:64] = head hl V for 4 key tiles.
            # j-outer: one open psum accumulation region per bank at a time.
            ps = pj.tile([128, 512], dt.float32, name="pjt", tag="pjt")
            for j in range(4):
                ks = 4 * kb + j
                for i in range(NI):
                    nc.tensor.matmul(
                        ps[:, DH * j:DH * (j + 1)],
                        lhsT=xv_sb[:, i, 128 * ks:128 * (ks + 1)],
                        rhs=wq_sb[:, i, 2 * DG + DH * hl:2 * DG + DH * (hl + 1)],
                        start=(i == 0), stop=(i == NI - 1))
            nc.vector.tensor_tensor(
                vext[:, 4 * kb:4 * kb + 4, hl, 0:DH], ps[:, 0:4 * DH],
                bvb_sb[:, :, DH * hl:DH * (hl + 1)], ALU.add)

        def gen_vh(hl):
            for kb in range(4):
                v_unit4(hl, kb)
                yield (f"vh{hl}", 4 * (kb + 1))

        def gen_tailk(k):
            # pass k's AV sweep + softmax normalize, absorbed into pass k+1.
            # q4-outer so each psum bank has ONE open accumulation region at a
            # time (hardware requirement); the two av banks run in parallel.
            # Progress label t{k}: count 16*q4 + ks + 1; ex[ks] is free once
            # count >= 48 + ks + 1 (its q4=3 read).
            if k == 0:
                for kb in range(4):
                    v_unit4(0, kb)
                    yield ("vh0", 4 * (kb + 1))
            av, exs, hl, qp = tails[k]
            for q4 in range(4):
                for ks in range(NKS):
                    nc.tensor.matmul(
                        av[0][:, q4, :],
                        lhsT=exs[ks][:, 128 * q4:128 * (q4 + 1)],
                        rhs=vext[:, ks, hl, :],
                        start=(ks == 0), stop=(ks == NKS - 1))
                    nc.tensor.matmul(
                        av[1][:, q4, :],
                        lhsT=exs[ks][:, 128 * (q4 + 4):128 * (q4 + 5)],
                        rhs=vext[:, ks, hl, :],
                        start=(ks == 0), stop=(ks == NKS - 1))
                    yield (f"t{k}", 16 * q4 + ks + 1)
            normalize(av, hl, qp, mix=(k == 7))
            yield (f"t{k}", 65)

        class Filler:
            def __init__(self, units):
                self.units = units
                self.done = {}
                self.empty = False
                self.gate = None       # blocked on this closed gate
                self.open = set()

            def pull(self, n=1):
                for _ in range(n):
                    if self.gate is not None:
                        if self.gate not in self.open:
                            return
                        self.gate = None
                    try:
                        lab = next(self.units)
                    except StopIteration:
                        self.empty = True
                        return
                    if lab is None:
                        continue
                    if lab[0] == "GATE":
                        if lab[1] not in self.open:
                            self.gate = lab[1]
                            return
                    else:
                        self.done[lab[0]] = lab[1]

            def open_gate(self, k):
                self.open.add(k)

            def require(self, key, count):
                while self.done.get(key, 0) < count:
                    assert not self.empty, f"filler exhausted before {key}={count}"
                    assert not (self.gate is not None and self.gate not in self.open), \
                        f"require {key}={count} blocked by gate {self.gate}"
                    self.pull(1)

        def gen_gate(k):
            yield ("GATE", k)

        def k_chunk(p, c):
            return gen_proj_chunk(xk_sb, kT[p], DG + 128 * p, 2 + p, f"k{p}", c)

        def q_chunk(p, c):
            return gen_proj_chunk(xq_sb, qT[p], 128 * p, p, f"q{p}", c)

        fill = Filler(chain(
            gen_warmup(10, dum[:, 0:128]),
            gen_warmup(15, wq_sb[:, 0, DG:DG + 128]),
            k_chunk(0, 0), q_chunk(0, 0), q_chunk(0, 1),
            k_chunk(0, 1), k_chunk(0, 2), k_chunk(0, 3),
            k_chunk(1, 0), k_chunk(1, 1), k_chunk(1, 2), k_chunk(1, 3),
            gen_gate(0), gen_tailk(0),
            gen_vh(1), q_chunk(1, 0), q_chunk(1, 1),
            gen_gate(1), gen_tailk(1),
            gen_vh(2), q_chunk(0, 2), q_chunk(0, 3),
            gen_gate(2), gen_tailk(2),
            gen_vh(3), q_chunk(1, 2), q_chunk(1, 3),
            gen_gate(3), gen_tailk(3),
            gen_gate(4), gen_tailk(4),
            gen_gate(5), gen_tailk(5),
            gen_gate(6), gen_tailk(6),
            gen_gate(7), gen_tailk(7),
        ))

        def emit_exp(qk, dve):
            if dve:
                exi = ex16s.tile([128, QP], dt.int16, name="exi", tag="exi")
                nc.vector.tensor_scalar(
                    exi[:], qk[:], A_DVE, B_DVE, ALU.mult, ALU.add)
                return exi.bitcast(dt.bfloat16)
            ext = exps.tile([128, QP], dt.bfloat16, name="ex", tag="ex")
            nc.scalar.activation(ext[:], qk[:], AF.Exp, scale=0.125)
            return ext

        def emit_qk(p, lo, qp, ks):
            qk = pp.tile([128, QP], dt.float32, name="qkt", tag="qkt")
            for nh in range(2):
                nc.tensor.matmul(
                    qk[:, 512 * nh:512 * (nh + 1)],
                    lhsT=kT[p][lo:lo + 64, 128 * ks:128 * (ks + 1)],
                    rhs=qT[p][lo:lo + 64, QP * qp + 512 * nh:QP * qp + 512 * nh + 512],
                    start=True, stop=True)
            return qk

        def attention_pass(idx, hl, qp):
            p, half = divmod(hl, 2)
            lo = 64 * half
            fill.require(f"q{p}", 2 * (qp + 1))
            av_lo = avp.tile([128, 4, DH + 1], dt.float32, name="av_lo", tag="av_lo")
            av_hi = avp.tile([128, 4, DH + 1], dt.float32, name="av_hi", tag="av_hi")
            exs = []
            tails[idx] = ([av_lo, av_hi], exs, hl, qp)
            for ks in range(NKS):
                fill.require(f"k{p}", min(ks // 4 + 1, 4))
                n = 16 * idx + ks
                if n - NEX >= 0:
                    ep, eks = divmod(n - NEX, NKS)
                    fill.require(f"t{ep}", 48 + eks + 1)
                qk = emit_qk(p, lo, qp, ks)
                exs.append(emit_exp(qk, idx in ALT_PASSES and ks % 2 == 1))
                fill.pull(5 if idx == 0 and ks >= 6 else 4)
            fill.open_gate(idx)

        order = [(0, 0), (1, 0), (2, 0), (3, 0), (0, 1), (1, 1), (2, 1), (3, 1)]
        for idx, (hl, qp) in enumerate(order):
            attention_pass(idx, hl, qp)
        while not fill.empty:
            fill.pull(4)

    nc.compile()
    return nc


def build_stage2(n_cores=8):
    nc = bacc.Bacc("TRN2", target_bir_lowering=False, debug=False, num_devices=n_cores)
    # xT[p, i, r] = x^T[128i + p, r] for this core's CHUNK rows
    xT = nc.declare_dram_parameter("xT", [128, NI, CHUNK], dt.bfloat16, isOutput=False)
    # woT[p, i, c] = Wo.T[128i + p, c]
    woT = nc.declare_dram_parameter("woT", [128, NI, D], dt.bfloat16, isOutput=False)
    bo = nc.declare_dram_parameter("bo", [1, D], dt.bfloat16, isOutput=False)
    out = nc.declare_dram_parameter("out", [CHUNK, D], dt.float32, isOutput=True)

    with tile.TileContext(nc) as tc, ExitStack() as ctx:
        pool = ctx.enter_context(tc.tile_pool(name="p2", bufs=1))
        ones_k1 = pool.tile([1, 512], dt.bfloat16, name="ones_k1", tag="ones")
        nc.vector.memset(ones_k1[:], 1.0)
        bo_sb = pool.tile([1, D], dt.bfloat16, name="bo_sb", tag="bo")
        nc.sync.dma_start(bo_sb[:], bo[:])
        xT_sb = pool.tile([128, NI, CHUNK], dt.bfloat16, name="xT_sb", tag="xT")
        wo_sb = pool.tile([128, NI, D], dt.bfloat16, name="wo_sb", tag="wo")
        for c in range(4):
            nc.sync.dma_start(xT_sb[:, 2 * c:2 * c + 2, :], xT[:, 2 * c:2 * c + 2, :])
            nc.sync.dma_start(wo_sb[:, 2 * c:2 * c + 2, :], woT[:, 2 * c:2 * c + 2, :])

        psp = ctx.enter_context(tc.tile_pool(name="psp", bufs=3, space="PSUM"))
        # PE p-state warmup while the first DMA chunks land
        for _ in range(14):
            ps = psp.tile([128, 512], dt.float32, name="fc", tag="fc")
            nc.tensor.matmul(ps[:], lhsT=ones_k1[0:1, 0:128], rhs=ones_k1[:],
                             start=True, stop=True)

        def mm(ps, g, t):
            rt, nh = divmod(g, 2)
            nc.tensor.matmul(
                ps[:], lhsT=xT_sb[:, t, 128 * rt:128 * (rt + 1)],
                rhs=wo_sb[:, t, 512 * nh:512 * (nh + 1)],
                start=(t == 0), stop=False)

        def finish(ps, g):
            rt, nh = divmod(g, 2)
            nc.tensor.matmul(
                ps[:], lhsT=ones_k1[0:1, 0:128],
                rhs=bo_sb[0:1, 512 * nh:512 * (nh + 1)],
                start=False, stop=True)
            os = pool.tile([128, 512], dt.float32, name=f"os{g}", tag=f"os{g % 4}",
                           bufs=1)
            if g % 2 == 0:
                nc.scalar.copy(os[:], ps[:])
            else:
                nc.vector.tensor_copy(os[:], ps[:])
            nc.sync.dma_start(
                out[128 * rt:128 * (rt + 1), 512 * nh:512 * (nh + 1)], os[:])

        # groups 0,1 interleaved across t (overlaps the input DMA window),
        # then groups 2..7 group-outer so output DMA overlaps compute
        ps0 = psp.tile([128, 512], dt.float32, name="fc", tag="fc")
        ps1 = psp.tile([128, 512], dt.float32, name="fc", tag="fc")
        for t in range(NI):
            mm(ps0, 0, t)
            mm(ps1, 1, t)
        finish(ps0, 0)
        finish(ps1, 1)
        for g in range(2, 8):
            ps = psp.tile([128, 512], dt.float32, name="fc", tag="fc")
            for t in range(NI):
                mm(ps, g, t)
            finish(ps, g)

    nc.compile()
    return nc


def _fold(a):
    """[Dfull, N] -> [128, Dfull//128, N] with element (p, i, n) = a[128i+p, n]."""
    d, n = a.shape
    return np.ascontiguousarray(a.reshape(d // 128, 128, n).transpose(1, 0, 2))


def stage1_inputs(inputs):
    """core c = (b, g): b = c // 4, g = c % 4 (head-group of 4 heads)."""
    xt = {}
    for nm in ('query', 'key', 'value'):
        for b in range(B):
            xt[(nm, b)] = _fold(np.ascontiguousarray(
                np.asarray(inputs[nm])[b].T).astype(BF16))
    maps = []
    for c in range(8):
        b, g = divmod(c, 4)
        rows = slice(DG * g, DG * (g + 1))
        bq = np.asarray(inputs['bq'])[rows].astype(np.float32)
        bk = np.asarray(inputs['bk'])[rows].astype(np.float32)
        bv = np.asarray(inputs['bv'])[rows].astype(np.float32)
        bqkT = np.stack([bq[0:128], bq[128:256], bk[0:128], bk[128:256]], axis=1)
        bvb = np.broadcast_to(bv[None, None, :], (128, 4, DG)).copy()
        w = np.concatenate([
            np.asarray(inputs['Wq'])[rows].T,
            np.asarray(inputs['Wk'])[rows].T,
            np.asarray(inputs['Wv'])[rows].T], axis=1).astype(BF16)
        maps.append({
            'xq': xt[('query', b)], 'xk': xt[('key', b)], 'xv': xt[('value', b)],
            'wqkv': _fold(np.ascontiguousarray(w)),
            'bqkT': bqkT, 'bvb': bvb,
        })
    return maps


def stage2_inputs(stage1_results, inputs):
    """stage1_results: list of 8 dicts with 'xo' [16, 128, 256] bf16 (normalized)."""
    woT = _fold(np.ascontiguousarray(np.asarray(inputs['Wo']).T).astype(BF16))
    bo = np.asarray(inputs['bo'])[None, :].astype(BF16)
    xrows_all = {}
    for cc in range(8):
        xrows_all[cc] = np.asarray(stage1_results[cc]['xo']).reshape(S, DG)
    maps = []
    for c in range(8):
        b, j = divmod(c, 4)
        rows = slice(CHUNK * j, CHUNK * (j + 1))
        xrows = np.concatenate(
            [xrows_all[4 * b + g][rows] for g in range(4)], axis=1)  # [512, 1024]
        maps.append({
            'xT': _fold(np.ascontiguousarray(xrows.T)),
            'woT': woT, 'bo': bo,
        })
    return maps


def assemble_output(stage2_results):
    out = np.empty((B, S, D), dtype=np.float32)
    for c in range(8):
        b, j = divmod(c, 4)
        out[b, CHUNK * j:CHUNK * (j + 1), :] = np.asarray(stage2_results[c]['out'])
    return out


_CACHE = {}


def _programs():
    if 'nc1' not in _CACHE:
        _CACHE['nc1'] = build_stage1(N_CORES)
        _CACHE['nc2'] = build_stage2(N_CORES)
    return _CACHE['nc1'], _CACHE['nc2']


def kernel(**inputs):
    nc1, nc2 = _programs()
    core_ids = list(range(N_CORES))
    s1_maps = stage1_inputs(inputs)
    r1 = run_bass_kernel_spmd(nc1, s1_maps, core_ids).results
    s2_maps = stage2_inputs(r1, inputs)
    r2 = run_bass_kernel_spmd(nc2, s2_maps, core_ids).results
    return assemble_output(r2)
